# revision 24
# baseline (speedup 1.0000x reference)
"""DolmaGPT (4-layer GPT, D=1024, H=16, T=1024, B=2, V=32000, ALiBi) on 8 TRN2 cores.

Strategy: sequence-parallel. Each core owns 256 token rows (cores 0-3 batch 0,
cores 4-7 batch 1). Weights replicated (bf16, streamed from HBM in pre-tiled
layouts so each load is one large-descriptor DMA). Per layer one fused 4-core
AllGather exchanges K^T and V together (bf16). lm_head vocab-sharded after an
8-core AllGather of the final hidden state. Residual stream fp32 in SBUF;
matmuls bf16 with fp32 PSUM accumulation.

v2 notes (instruction-count + pipelining oriented):
- Weights pre-tiled on host to [P, KC, F] so each load is one DMA with 2KB
  descriptors (v1 used 16-32 small strided DMAs per weight at 256B/desc).
- LN transposes via the DMA XBAR (dma_start_transpose) instead of PE
  transpose + copy chains.
- PSUM tiles span 4 banks' worth of chunks ([P, 4, NTOK]) so exp / gelu /
  PSUM->SBUF copies are one instruction per 4 matmul groups.
- K and V share one AllGather message per layer.
- lm_head processes vocab chunks in pairs per PSUM tile; output stored bf16.
- Softmax denominator broadcast on gpsimd (partition_broadcast), psy PSUM
  double-buffered: successive attention heads pipeline (this was worth ~2x
  on measured HW time).
- Output stores on the Act HWDGE queue. NOTE: gpsimd/SWDGE dma_start stores
  crashed the device (NRT_EXEC_UNIT_UNRECOVERABLE) alongside collectives.

Softmax: scores bounded, so no max-subtract. P = exp(scale*s) * M where
M = exp(alibi_bias) (0 where masked) is a precomputed per-core constant.
Denominator via ones-column appended to V (one extra PSUM row per head).
"""
import contextlib
import math
import numpy as np
import ml_dtypes

import concourse.bacc as bacc
import concourse.bass as bass
import concourse.mybir as mybir
import concourse.tile as tile
from concourse.bass import ts, ds
from concourse.masks import make_identity

P = 128
HD = 64
EPS = 1e-5
ALIBI_BIAS_MAX = 8.0
NCORES = 8
GS = 4  # AllGather group size for K/V (cores sharing one batch element)

FULL = dict(V=32000, D=1024, H=16, L=4, F=4096, B=2, T=1024,
            store_act=True)

F32 = mybir.dt.float32
BF16 = mybir.dt.bfloat16
FP8 = mybir.dt.float8e4
I32 = mybir.dt.int32
FP8_WS = 16.0   # fp8 MLP weight scale (values ~N(0,.02) -> normal range)


def build_program(cfg):
    V, D, H, L, F, B, T = (cfg[k] for k in ("V", "D", "H", "L", "F", "B", "T"))
    NTOK = B * T // NCORES      # tokens per core
    QT = NTOK // P              # q-token tiles per core
    KC = D // P                 # contract chunks over D
    KT = T // P                 # k-token tiles (attention keys, own batch)
    FT = F // P                 # MLP hidden tiles
    FC = F // P                 # fc2 contract chunks
    VS = V // NCORES            # vocab shard per core
    NV = 500 if VS % 1000 == 0 else (128 if VS % 256 == 0 else VS)
    NVC = VS // NV
    PAIR = 2 if NVC % 2 == 0 else 1
    NVC2 = NVC // PAIR          # fused weight chunks of PAIR*NV columns
    MT = B * T // P             # global token tiles (lm_head rows)
    scale = 1.0 / math.sqrt(HD)
    NCH = min(512, D)           # N-chunk for [tok, feat] matmuls
    WF = min(1024, D)           # feature width of one weight tile
    XX = D // NTOK if D >= NTOK else 0   # v-export row split (see kvmsg)
    H2 = NTOK // HD             # heads per 256-col row-chunk of v region
    ZB = bool(cfg.get("zero_bias"))  # skip K=1 bias matmuls when biases zero
    FP8M = bool(cfg.get("fp8_mlp"))  # fc/fc2 in fp8e4 DoubleRow (needs ZB)
    STORE_ACT = bool(cfg.get("store_act"))   # osb stores via Act HWDGE
    NO_XBAR = bool(cfg.get("no_xbar"))       # PE transposes instead of XBAR
    SP_DMA = bool(cfg.get("sp_dma"))         # all loads via SP queue
    Q4 = 4                      # chunk group per PSUM tile

    assert D % NTOK == 0 and H == (D // NTOK) * (NTOK // HD)
    assert H % 4 == 0 or H == 4

    nc = bacc.Bacc("TRN2", target_bir_lowering=False, debug=False,
                   num_devices=NCORES)
    eng2 = nc.sync if SP_DMA else nc.scalar   # gathered acts / masks queue

    # ---- DRAM parameters (identical shapes on every core) ----
    # pre-tiled weights: [.., P, kc, feat] so one load = one big-desc DMA
    ids_in = nc.declare_dram_parameter("ids", [QT, P], I32, isOutput=False)
    wte_in = nc.declare_dram_parameter("wte", [V, D], F32, isOutput=False)
    # slot 0 = K feats, slot 1 = V feats, slot 2 = Q feats
    wkvq_in = nc.declare_dram_parameter("wkvq", [L, 3, P, KC, D], BF16,
                                        isOutput=False)
    wproj_in = nc.declare_dram_parameter("wprojT", [L, P, KC, D], BF16,
                                         isOutput=False)
    MDT = FP8 if FP8M else BF16
    wfc_in = nc.declare_dram_parameter("wfcT", [L, P, KC, F], MDT,
                                       isOutput=False)
    wfc2_in = nc.declare_dram_parameter("wfc2T", [L, P, FC, D], MDT,
                                        isOutput=False)
    cm_in = nc.declare_dram_parameter("cm", [T, NTOK], F32, isOutput=False)
    augq_in = nc.declare_dram_parameter("augq", [2, H, NTOK], mybir.dt.float16,
                                        isOutput=False)
    augk_in = nc.declare_dram_parameter("augk", [2, T], mybir.dt.float16,
                                        isOutput=False)
    wlm_in = nc.declare_dram_parameter("wlmT", [NVC2, P, KC, PAIR * NV], BF16,
                                       isOutput=False)
    if not ZB:
        qkb_in = nc.declare_dram_parameter("qkb", [L, 2 * D], F32, isOutput=False)
        vb_in = nc.declare_dram_parameter("vb", [L, D], F32, isOutput=False)
        pb_in = nc.declare_dram_parameter("pb", [L, D], F32, isOutput=False)
        fcb_in = nc.declare_dram_parameter("fcb", [L, F], F32, isOutput=False)
        f2b_in = nc.declare_dram_parameter("f2b", [L, D], F32, isOutput=False)
        lmb_in = nc.declare_dram_parameter("lmb", [VS], F32, isOutput=False)
    out_d = nc.declare_dram_parameter("out", [B * T, VS], BF16, isOutput=True)

    HG = min(H, 4)              # heads per mask tile
    NHG = H // HG

    with tile.TileContext(nc) as tc:
        with (
            tc.tile_pool(name="const", bufs=1) as constp,
            tc.tile_pool(name="resident", bufs=1) as resp,
            tc.tile_pool(name="acts", bufs=1) as actp,
            tc.tile_pool(name="w", bufs=3) as wp_,
            tc.tile_pool(name="ln", bufs=2) as lnp,
            tc.tile_pool(name="stats", bufs=4) as statp,
            tc.tile_pool(name="mask", bufs=2) as maskp,
            tc.tile_pool(name="softmax", bufs=2) as softp,
            tc.tile_pool(name="outcp", bufs=2) as outp,
            tc.tile_pool(name="psA", bufs=(2 if NO_XBAR else 3),
                         space="PSUM") as psA,
            tc.tile_pool(name="psY", bufs=2, space="PSUM") as psY,
            tc.tile_pool(name="dram", bufs=2, space="DRAM") as dramp,
            contextlib.ExitStack() as estack,
        ):
            psT = (estack.enter_context(
                tc.tile_pool(name="psT", bufs=2, space="PSUM"))
                if NO_XBAR else None)
            # ---- constants ----
            if NO_XBAR:
                id_bf = constp.tile([P, P], BF16)
                make_identity(nc, id_bf[:, :])
            ones_bank = constp.tile([65, P], F32)
            nc.vector.memset(ones_bank[:, :], 1.0)
            eps_sb = constp.tile([P, 1], F32)
            nc.vector.memset(eps_sb[:, :], EPS)

            if not ZB:
                QKC = 2 * D // P
                qkb_sb = constp.tile([P, L * QKC], F32)
                nc.scalar.dma_start(
                    out=qkb_sb[:, :],
                    in_=qkb_in[:, :].rearrange("l (c p) -> p (l c)", p=P))
                fcb_sb = constp.tile([P, L * FT], F32)
                nc.scalar.dma_start(
                    out=fcb_sb[:, :],
                    in_=fcb_in[:, :].rearrange("l (c p) -> p (l c)", p=P))
                rows_sb = constp.tile([65, L * D], F32)
                nc.scalar.dma_start(
                    out=rows_sb[0:1, 0:L * D],
                    in_=vb_in[:, :].rearrange("l d -> (l d)")
                    .rearrange("(o f) -> o f", o=1))
                nc.scalar.dma_start(
                    out=rows_sb[32:33, 0:L * D],
                    in_=pb_in[:, :].rearrange("l d -> (l d)")
                    .rearrange("(o f) -> o f", o=1))
                nc.scalar.dma_start(
                    out=rows_sb[64:65, 0:L * D],
                    in_=f2b_in[:, :].rearrange("l d -> (l d)")
                    .rearrange("(o f) -> o f", o=1))
                vb_sb = rows_sb[0:1, :]
                pb_sb = rows_sb[32:33, :]
                f2b_sb = rows_sb[64:65, :]

            # ---- ALiBi folded into QK via 2 extra contraction rows:
            # psum += augk^T @ augq_h gives slope_h*(k - q)/scale exactly
            # (per-column f16 rounding cancels in softmax). Cm is 0 on
            # valid (k<=q) entries, -BIG on masked ones, added pre-exp. ----
            cm_sb = resp.tile([P, KT, NTOK], F32)
            eng2.dma_start(out=cm_sb[:, :, :],
                           in_=cm_in[:, :].rearrange("(kc p) q -> p kc q",
                                                     p=P))
            augq_sb = resp.tile([2, H * NTOK], mybir.dt.float16)
            eng2.dma_start(out=augq_sb[:, :],
                           in_=augq_in[:, :, :].rearrange("x h q -> x (h q)"))
            augk_sb = resp.tile([2, T], mybir.dt.float16)
            eng2.dma_start(out=augk_sb[:, :], in_=augk_in[:, :])

            # ---- residual stream x [p, a, d], fp32, resident ----
            x = resp.tile([P, QT, D], F32)
            for a in range(QT):
                ids_sb = statp.tile([P, 1], I32, tag="ids")
                nc.sync.dma_start(out=ids_sb[:, :],
                                  in_=ids_in[a, :].rearrange("(p o) -> p o", o=1))
                nc.gpsimd.indirect_dma_start(
                    out=x[:, a, :], out_offset=None,
                    in_=wte_in[:, :],
                    in_offset=bass.IndirectOffsetOnAxis(ap=ids_sb[:, 0:1], axis=0))

            # ---- v_aug resident [p, kt, h*(HD+1)+j]; ones col per head ----
            v_aug = resp.tile([P, KT, H * (HD + 1)], BF16)
            va4 = v_aug.rearrange("p k (h j) -> p k h j", j=HD + 1)
            nc.vector.memset(va4[:, :, :, HD:HD + 1], 1.0)



            def layer_norm_T(src, zT, a_iter=None):
                """LN over free dim of src [P, QT, D] -> transposed bf16 zT
                [P, KC, NTOK] via the DMA XBAR."""
                for a in (range(QT) if a_iter is None else a_iter):
                    xs = src[:, a, :]
                    sm = statp.tile([P, 1], F32, tag="lnsm")
                    scr = lnp.tile([P, D], BF16, tag="lnscr", bufs=2)
                    nc.scalar.activation(scr[:, :], xs,
                                         mybir.ActivationFunctionType.Identity,
                                         accum_out=sm[:, :])
                    sqd = lnp.tile([P, D], BF16, tag="lnsqd", bufs=2)
                    sx2 = statp.tile([P, 1], F32, tag="lnsx2")
                    nc.scalar.activation(sqd[:, :], xs,
                                         mybir.ActivationFunctionType.Square,
                                         accum_out=sx2[:, :])
                    mean = statp.tile([P, 1], F32, tag="lnmean")
                    nc.scalar.mul(mean[:, :], sm[:, :], 1.0 / D)
                    nmean = statp.tile([P, 1], F32, tag="lnnmean")
                    nc.scalar.mul(nmean[:, :], sm[:, :], -1.0 / D)
                    b2 = statp.tile([P, 1], F32, tag="lnb2")
                    nc.vector.tensor_scalar(out=b2[:, :], in0=mean[:, :],
                                            scalar1=nmean[:, :],
                                            scalar2=eps_sb[:, :],
                                            op0=mybir.AluOpType.mult,
                                            op1=mybir.AluOpType.add)
                    std = statp.tile([P, 1], F32, tag="lnstd")
                    nc.scalar.activation(std[:, :], sx2[:, :],
                                         mybir.ActivationFunctionType.Sqrt,
                                         bias=b2[:, :], scale=1.0 / D)
                    rstd = statp.tile([P, 1], F32, tag="lnrstd")
                    nc.vector.reciprocal(rstd[:, :], std[:, :])
                    mrstd = statp.tile([P, 1], F32, tag="lnmrstd")
                    nc.vector.tensor_mul(out=mrstd[:, :], in0=mean[:, :],
                                         in1=rstd[:, :])
                    z = lnp.tile([P, D], BF16, tag="lnz", bufs=2)
                    nc.vector.tensor_scalar(out=z[:, :], in0=xs,
                                            scalar1=rstd[:, :],
                                            scalar2=mrstd[:, :],
                                            op0=mybir.AluOpType.mult,
                                            op1=mybir.AluOpType.subtract)
                    # zT[p, kc, t] = z[t, kc*128+p]
                    if NO_XBAR:
                        for kc in range(KC):
                            pt = psT.tile([P, P], BF16, tag="tr", bufs=2)
                            nc.tensor.transpose(out=pt[:, :],
                                                in_=z[:, ts(kc, P)],
                                                identity=id_bf[:, :])
                            nc.scalar.copy(out=zT[:, kc, ts(a, P)],
                                           in_=pt[:, :])
                    else:
                        nc.sync.dma_start_transpose(out=zT[:, :, ts(a, P)],
                                                    in_=z[:, :])

            def load_w(src_ap, name, dt=BF16):
                """One weight tile [P, KC, <=WF] from a pre-tiled layout.
                All tiles share one uniformly-shaped pool slot."""
                wt = wp_.tile([P, KC, WF], dt, tag="w", bufs=3, name=name)
                nc.sync.dma_start(out=wt[:, :, 0:src_ap.shape[2]], in_=src_ap)
                return wt

            def mm_quad(dst, wtile, h_src, c0, ncnk, l, bias_sb=None, boff=0,
                        act=None):
                """ncnk feature chunks [feat,tok] into one PSUM tile, then one
                batched copy/activation to dst[:, c0:c0+ncnk, :]."""
                ps = psA.tile([P, Q4, NTOK], F32, tag="quad")
                for j in range(ncnk):
                    for kc in range(KC):
                        nc.tensor.matmul(out=ps[:, j, :],
                                         lhsT=wtile[:, kc, ts(c0 + j, P)],
                                         rhs=h_src[:, kc, :],
                                         start=(kc == 0), stop=(kc == KC - 1))
                fn = act or mybir.ActivationFunctionType.Identity
                if ZB or bias_sb is None:
                    if act is None:
                        nc.scalar.copy(out=dst[:, c0:c0 + ncnk, :],
                                       in_=ps[:, 0:ncnk, :])
                    else:
                        nc.scalar.activation(dst[:, c0:c0 + ncnk, :],
                                             ps[:, 0:ncnk, :], fn)
                else:
                    for j in range(ncnk):
                        nc.scalar.activation(
                            dst[:, c0 + j, :], ps[:, j, :], fn,
                            bias=bias_sb[:, boff + c0 + j: boff + c0 + j + 1])

            nn0 = D // NCH
            AOFF = max(NCH, 512)    # bank-separate concurrent accum groups
            NKH = min(512, T)       # psum free width for kT_full
            assert QT == 2
            HPC = NCH // HD          # heads per v psum copy chunk
            for l in range(L):
                # ===== LN1 -> h1T, one token-half at a time; each half's
                # h is AllGathered separately so KV compute on half A
                # overlaps half B's collective =====
                h1T = actp.tile([P, KC, NTOK], BF16, tag="hT", bufs=1)
                SPLIT = bool(cfg.get("split_cc"))
                hgs = []
                if SPLIT:
                    for half in range(2):
                        layer_norm_T(x, h1T, a_iter=[half])
                        hmsg = dramp.tile([D, P], BF16, tag="kvmsg")
                        nc.sync.dma_start(
                            out=hmsg[:, :].rearrange("(kc p) q -> p kc q", p=P),
                            in_=h1T[:, :, ts(half, P)])
                        hg = dramp.tile([GS * D, P], BF16, tag="kvg")
                        if cfg.get("no_cc"):
                            for g in range(GS):
                                nc.sync.dma_start(out=hg[ts(g, D), :],
                                                  in_=hmsg[:, :])
                        else:
                            nc.gpsimd.collective_compute(
                                "AllGather", mybir.AluOpType.bypass,
                                ins=[hmsg[:, :].opt()], outs=[hg[:, :].opt()],
                                replica_groups=[[0, 1, 2, 3], [4, 5, 6, 7]])
                        hgs.append(hg)
                else:
                    layer_norm_T(x, h1T)
                    hmsg = dramp.tile([D, NTOK], BF16, tag="kvmsg")
                    nc.sync.dma_start(
                        out=hmsg[:, :].rearrange("(kc p) q -> p kc q", p=P),
                        in_=h1T[:, :, :])
                    hg = dramp.tile([GS * D, NTOK], BF16, tag="kvg")
                    if cfg.get("no_cc"):
                        for g in range(GS):
                            nc.sync.dma_start(out=hg[ts(g, D), :],
                                              in_=hmsg[:, :])
                    else:
                        nc.gpsimd.collective_compute(
                            "AllGather", mybir.AluOpType.bypass,
                            ins=[hmsg[:, :].opt()], outs=[hg[:, :].opt()],
                            replica_groups=[[0, 1, 2, 3], [4, 5, 6, 7]])

                wq = load_w(wkvq_in[l, 2, :, :, :], f"wq{l}")
                wk = load_w(wkvq_in[l, 0, :, :, :], f"wk{l}")
                wv = load_w(wkvq_in[l, 1, :, :, :], f"wv{l}")

                # ===== qT (local tokens; overlaps the AllGathers) =====
                qT = actp.tile([P, KC, NTOK], BF16, tag="qT", bufs=1)
                for c0 in range(0, KC, Q4):
                    n = min(Q4, KC - c0)
                    mm_quad(qT, wq, h1T, c0, n, l,
                            bias_sb=None if ZB else qkb_sb,
                            boff=None if ZB else l * (2 * D // P))

                # ===== per half: unpack h, kT chunks, v tiles =====
                hg_sb = actp.tile([P, KC, T], BF16, tag="hg", bufs=1)
                hgv = hg_sb.rearrange("p k (g x q) -> p k g x q", x=2, q=P)
                kT = actp.tile([P, KC, T], BF16, tag="big32", bufs=1)
                kTv = kT.rearrange("p c (g x q) -> p c g x q", x=2, q=P)
                if not SPLIT:
                    for g in range(GS):
                        eng2.dma_start(
                            out=hg_sb[:, :, ts(g, NTOK)],
                            in_=hg[g * D:(g + 1) * D, :]
                            .rearrange("(kc p) q -> p kc q", p=P))
                for half in range(2):
                    if SPLIT:
                        for g in range(GS):
                            eng2.dma_start(
                                out=hgv[:, :, g, half, :],
                                in_=hgs[half][g * D:(g + 1) * D, :]
                                .rearrange("(kc p) q -> p kc q", p=P))
                    # kT chunks for this half's keys (all 4 cores' blocks)
                    for c in range(KC):
                        ps = psA.tile([P, Q4, NTOK], F32, tag="quad")
                        pso = ps.rearrange("p a q -> p (a q)")[:, 0:GS * P] \
                            .rearrange("p (g q) -> p g q", q=P)
                        for kc in range(KC):
                            nc.tensor.matmul(
                                out=pso[:, :, :],
                                lhsT=wk[:, kc, ts(c, P)],
                                rhs=hgv[:, kc, :, half, :],
                                start=(kc == 0), stop=(kc == KC - 1))
                        if ZB:
                            nc.scalar.copy(out=kTv[:, c, :, half, :],
                                           in_=pso[:, :, :])
                        else:
                            nc.scalar.activation(
                                kTv[:, c, :, half, :], pso[:, :, :],
                                mybir.ActivationFunctionType.Identity,
                                bias=qkb_sb[:, l * (2 * D // P) + KC + c:
                                            l * (2 * D // P) + KC + c + 1])
                    # v tiles for this half (jk % 2 == half)
                    for jk in range(half, KT, 2):
                        ps = psA.tile([P, Q4, NTOK], F32, tag="quad")
                        psf = ps.rearrange("p a q -> p (a q)")
                        for kc in range(KC):
                            for n0 in range(nn0):
                                nc.tensor.matmul(
                                    out=psf[:, n0 * AOFF:n0 * AOFF + NCH],
                                    lhsT=hg_sb[:, kc, ts(jk, P)],
                                    rhs=wv[:, kc, ts(n0, NCH)],
                                    start=(kc == 0),
                                    stop=(ZB and kc == KC - 1))
                        if not ZB:
                            for n0 in range(nn0):
                                nc.tensor.matmul(
                                    out=psf[:, n0 * AOFF:n0 * AOFF + NCH],
                                    lhsT=ones_bank[0:1, 0:P],
                                    rhs=vb_sb[:, l * D + n0 * NCH:
                                              l * D + n0 * NCH + NCH],
                                    start=False, stop=True)
                        for n0 in range(nn0):
                            nc.scalar.copy(
                                out=va4[:, jk, n0 * HPC:(n0 + 1) * HPC, 0:HD],
                                in_=psf[:, n0 * AOFF:n0 * AOFF + NCH]
                                .rearrange("p (h j) -> p h j", j=HD))

                wproj = load_w(wproj_in[l, :, :, :], f"wp{l}")
                nwf = (F + WF - 1) // WF  # fc weight tiles (feature-split)
                wfcs = [load_w(wfc_in[l, :, :, ts(i, WF)], f"wfc{l}_{i}",
                               dt=MDT) for i in range(min(2, nwf))]

                # ===== attention per head =====
                yT = actp.tile([P, KC, NTOK], BF16, tag="yT", bufs=1)
                for h in range(H):
                    hr = (h % 2) * HD
                    hc = h // 2
                    p_all = softp.tile([P, KT, NTOK], BF16, tag="p_all", bufs=2)
                    for j0 in range(0, KT, Q4):
                        nq = min(Q4, KT - j0)
                        pss = psA.tile([P, Q4, NTOK], F32, tag="quad")
                        for jj in range(nq):
                            jk = j0 + jj
                            nc.tensor.matmul(
                                out=pss[:, jj, :],
                                lhsT=kT[hr:hr + HD, hc, ts(jk, P)],
                                rhs=qT[hr:hr + HD, hc, :],
                                start=True, stop=False)
                            nc.tensor.matmul(
                                out=pss[:, jj, :],
                                lhsT=augk_sb[0:2, ts(jk, P)],
                                rhs=augq_sb[0:2, h * NTOK:(h + 1) * NTOK],
                                start=False, stop=True)
                        nc.vector.tensor_add(out=pss[:, 0:nq, :],
                                             in0=pss[:, 0:nq, :],
                                             in1=cm_sb[:, j0:j0 + nq, :])
                        nc.scalar.activation(p_all[:, j0:j0 + nq, :],
                                             pss[:, 0:nq, :],
                                             mybir.ActivationFunctionType.Exp,
                                             scale=scale)
                    psy = psY.tile([HD + 1, NTOK], F32, tag="psy", bufs=2)
                    for jk in range(KT):
                        nc.tensor.matmul(
                            out=psy[:, :],
                            lhsT=v_aug[:, jk, h * (HD + 1):(h + 1) * (HD + 1)],
                            rhs=p_all[:, jk, :],
                            start=(jk == 0), stop=(jk == KT - 1))
                    rden = statp.tile([1, NTOK], F32, tag="rden")
                    nc.vector.reciprocal(rden[:, :], psy[HD:HD + 1, :])
                    rbc = softp.tile([HD, NTOK], F32, tag="rbc", bufs=2)
                    nc.gpsimd.partition_broadcast(rbc[:, :], rden[:, :],
                                                  channels=HD)
                    nc.vector.tensor_mul(out=yT[hr:hr + HD, hc, :],
                                         in0=psy[0:HD, :], in1=rbc[:, :])

                wfcs += [load_w(wfc_in[l, :, :, ts(i, WF)], f"wfc{l}_{i}",
                                dt=MDT) for i in range(2, nwf)]

                # ===== proj + residual (lhsT reused across n0 chunks) =====
                for a in range(QT):
                    ps = psA.tile([P, Q4, NTOK], F32, tag="quad")
                    psf = ps.rearrange("p a q -> p (a q)")
                    for kc in range(KC):
                        for n0 in range(nn0):
                            nc.tensor.matmul(
                                out=psf[:, n0 * AOFF:n0 * AOFF + NCH],
                                lhsT=yT[:, kc, ts(a, P)],
                                rhs=wproj[:, kc, ts(n0, NCH)],
                                start=(kc == 0),
                                stop=(ZB and kc == KC - 1))
                    if not ZB:
                        for n0 in range(nn0):
                            nc.tensor.matmul(
                                out=psf[:, n0 * AOFF:n0 * AOFF + NCH],
                                lhsT=ones_bank[32:33, 0:P],
                                rhs=pb_sb[:, l * D + n0 * NCH:
                                          l * D + n0 * NCH + NCH],
                                start=False, stop=True)
                    if AOFF == NCH:
                        nc.vector.tensor_add(out=x[:, a, :], in0=x[:, a, :],
                                             in1=psf[:, 0:D])
                    else:
                        for n0 in range(nn0):
                            nc.vector.tensor_add(
                                out=x[:, a, ts(n0, NCH)],
                                in0=x[:, a, ts(n0, NCH)],
                                in1=psf[:, n0 * AOFF:n0 * AOFF + NCH])

                # ===== LN2 -> h2T; MLP =====
                h2T = actp.tile([P, KC, NTOK], BF16, tag="hT", bufs=1)
                layer_norm_T(x, h2T)

                if FP8M:
                    assert ZB and KC % 2 == 0
                    h2T8 = actp.tile([P, KC, NTOK], FP8, tag="qT8", bufs=1)
                    nc.scalar.copy(out=h2T8[:, :, :], in_=h2T[:, :, :])
                gT = actp.tile([P, FT, NTOK], MDT, tag="big32", bufs=1)
                FPW = WF // P  # feature chunks per fc tile
                FQ = min(Q4, FPW)
                DR = mybir.MatmulPerfMode.DoubleRow
                for c0 in range(0, FT, FQ):
                    n = min(FQ, FT - c0)
                    wt = wfcs[c0 // FPW]
                    ps = psA.tile([P, Q4, NTOK], F32, tag="quad")
                    for j in range(n):
                        loc = (c0 + j) - (c0 // FPW) * FPW
                        if FP8M:
                            for kp in range(KC // 2):
                                nc.tensor.matmul(
                                    out=ps[:, j, :],
                                    lhsT=wt[:, 2 * kp:2 * kp + 2, ts(loc, P)],
                                    rhs=h2T8[:, 2 * kp:2 * kp + 2, :],
                                    perf_mode=DR,
                                    start=(kp == 0), stop=(kp == KC // 2 - 1))
                        else:
                            for kc in range(KC):
                                nc.tensor.matmul(out=ps[:, j, :],
                                                 lhsT=wt[:, kc, ts(loc, P)],
                                                 rhs=h2T[:, kc, :],
                                                 start=(kc == 0),
                                                 stop=(kc == KC - 1))
                    gsc = (1.0 / FP8_WS) if FP8M else 1.0
                    if ZB:
                        nc.scalar.activation(gT[:, c0:c0 + n, :], ps[:, 0:n, :],
                                             mybir.ActivationFunctionType.Gelu,
                                             scale=gsc)
                    else:
                        for j in range(n):
                            nc.scalar.activation(
                                gT[:, c0 + j, :], ps[:, j, :],
                                mybir.ActivationFunctionType.Gelu,
                                bias=fcb_sb[:, l * FT + c0 + j:
                                            l * FT + c0 + j + 1])

                # ===== fc2: accumulate over FC chunks (lhsT reused over n0) =====
                FB = KC if FC % KC == 0 else min(8, FC)
                w2s = [load_w(wfc2_in[l, :, ts(i, FB), :], f"wfc2{l}_{i}",
                              dt=MDT) for i in range(FC // FB)]
                q4s = [psA.tile([P, Q4, NTOK], F32, tag="quad",
                                name=f"fc2q{l}_{a}") for a in range(QT)]
                for fb in range(FC // FB):
                    w2 = w2s[fb]
                    for a in range(QT):
                        q4f = q4s[a].rearrange("p a q -> p (a q)")
                        if FP8M:
                            assert FB % 2 == 0
                            for jp in range(FB // 2):
                                j2 = fb * FB + 2 * jp
                                for n0 in range(nn0):
                                    nc.tensor.matmul(
                                        out=q4f[:, n0 * AOFF:n0 * AOFF + NCH],
                                        lhsT=gT[:, j2:j2 + 2, ts(a, P)],
                                        rhs=w2[:, 2 * jp:2 * jp + 2,
                                               ts(n0, NCH)],
                                        perf_mode=DR,
                                        start=(fb == 0 and jp == 0 and False
                                               or (fb == 0 and jp == 0)),
                                        stop=(fb == FC // FB - 1
                                              and jp == FB // 2 - 1))
                        else:
                            for j in range(FB):
                                for n0 in range(nn0):
                                    nc.tensor.matmul(
                                        out=q4f[:, n0 * AOFF:n0 * AOFF + NCH],
                                        lhsT=gT[:, fb * FB + j, ts(a, P)],
                                        rhs=w2[:, j, ts(n0, NCH)],
                                        start=(fb == 0 and j == 0),
                                        stop=(ZB and fb == FC // FB - 1
                                              and j == FB - 1))
                for a in range(QT):
                    q4f = q4s[a].rearrange("p a q -> p (a q)")
                    if not ZB:
                        for n0 in range(nn0):
                            nc.tensor.matmul(
                                out=q4f[:, n0 * AOFF:n0 * AOFF + NCH],
                                lhsT=ones_bank[64:65, 0:P],
                                rhs=f2b_sb[:, l * D + n0 * NCH:
                                           l * D + n0 * NCH + NCH],
                                start=False, stop=True)
                    if FP8M:
                        m2 = lnp.tile([P, D], F32, tag="m2", bufs=2)
                        nc.scalar.activation(
                            m2[:, :], q4f[:, 0:D],
                            mybir.ActivationFunctionType.Identity,
                            scale=1.0 / FP8_WS)
                        nc.vector.tensor_add(out=x[:, a, :], in0=x[:, a, :],
                                             in1=m2[:, :])
                    elif AOFF == NCH:
                        nc.vector.tensor_add(out=x[:, a, :], in0=x[:, a, :],
                                             in1=q4f[:, 0:D])
                    else:
                        for n0 in range(nn0):
                            nc.vector.tensor_add(
                                out=x[:, a, ts(n0, NCH)],
                                in0=x[:, a, ts(n0, NCH)],
                                in1=q4f[:, n0 * AOFF:n0 * AOFF + NCH])

            # ===== final LN -> xfT; AllGather over all 8 cores;
            # lm_head vocab-sharded (8MB weights/core) =====
            wl0 = load_w(wlm_in[0, :, :, :], "wl0")
            xfT = actp.tile([P, KC, NTOK], BF16, tag="hT", bufs=1)
            layer_norm_T(x, xfT)
            xfmsg = dramp.tile([D, NTOK], BF16, tag="kvmsg")
            nc.sync.dma_start(
                out=xfmsg[:, :].rearrange("(kc p) q -> p kc q", p=P),
                in_=xfT[:, :, :])
            if cfg.get("no_cc"):
                xfg = dramp.tile([NCORES * D, NTOK], BF16, tag="xfg")
                for g in range(NCORES):
                    nc.sync.dma_start(out=xfg[ts(g, D), :], in_=xfmsg[:, :])
            else:
                xfg = dramp.tile([NCORES * D, NTOK], BF16, tag="xfg",
                                 addr_space="Shared")
                nc.gpsimd.collective_compute(
                    "AllGather", mybir.AluOpType.bypass,
                    ins=[xfmsg[:, :].opt()], outs=[xfg[:, :].opt()],
                    replica_groups=[list(range(NCORES))])
            # overlays the (now dead) gathered-h slot to fit SBUF
            xfg_sb = actp.tile([P, NCORES * KC, NTOK], BF16, tag="hg",
                               bufs=1)
            eng2.dma_start(
                out=xfg_sb[:, :, :],
                in_=xfg[:, :].rearrange("(c p) t -> p c t", p=P))

            for j in range(NVC2):
                wls = wl0 if j == 0 else load_w(wlm_in[j, :, :, :], f"wl{j}")
                if not ZB:
                    lmb_t = statp.tile([1, PAIR * NV], F32, tag="lmbt", bufs=2)
                    nc.scalar.dma_start(
                        out=lmb_t[:, :],
                        in_=lmb_in[j * PAIR * NV:(j + 1) * PAIR * NV]
                        .rearrange("(o v) -> o v", o=1))
                for m in range(MT):
                    g, i = m // QT, m % QT
                    ps = psA.tile([P, Q4, NTOK], F32, tag="quad")
                    psv = ps.rearrange("p a q -> p (a q)").rearrange(
                        "p (s v) -> p s v", s=PAIR)
                    for kc in range(KC):
                        for s in range(PAIR):
                            nc.tensor.matmul(
                                out=psv[:, s, 0:NV],
                                lhsT=xfg_sb[:, g * KC + kc, ts(i, P)],
                                rhs=wls[:, kc, s * NV:s * NV + NV],
                                start=(kc == 0),
                                stop=(ZB and kc == KC - 1))
                    if not ZB:
                        for s in range(PAIR):
                            nc.tensor.matmul(
                                out=psv[:, s, 0:NV],
                                lhsT=ones_bank[0:1, 0:P],
                                rhs=lmb_t[:, s * NV:(s + 1) * NV],
                                start=False, stop=True)
                    osb = outp.tile([P, PAIR, NV], BF16, tag="osb",
                                    bufs=(1 if NO_XBAR else 2))
                    nc.scalar.copy(out=osb[:, :, :], in_=psv[:, :, 0:NV])
                    (nc.scalar if STORE_ACT else nc.gpsimd).dma_start(
                        out=out_d[ts(m, P), j * PAIR * NV:(j + 1) * PAIR * NV],
                        in_=osb[:, :, :])

    nc.finalize()
    return nc


# ---------------- host side ----------------

def _bf16(a):
    return np.asarray(a, dtype=ml_dtypes.bfloat16)


def _fp8(a):
    return np.asarray(a, dtype=ml_dtypes.float8_e4m3)


def prep_inputs(cfg, inputs):
    """Build the 8 per-core input maps from full inputs (pre-tiled layouts)."""
    V, D, H, L, F, B, T = (cfg[k] for k in ("V", "D", "H", "L", "F", "B", "T"))
    NTOK = B * T // NCORES
    KC = D // P
    FC = F // P
    VS = V // NCORES
    NV = 500 if VS % 1000 == 0 else (128 if VS % 256 == 0 else VS)
    NVC = VS // NV
    PAIR = 2 if NVC % 2 == 0 else 1
    NVC2 = NVC // PAIR

    ids = np.asarray(inputs["input_ids"]).astype(np.int32).reshape(-1)
    wte = np.asarray(inputs["wte"], dtype=np.float32)
    ln1_g = np.asarray(inputs["ln1_g"], np.float32)
    ln1_b = np.asarray(inputs["ln1_b"], np.float32)
    attn_w = np.asarray(inputs["attn_w"], np.float32)
    attn_b = np.asarray(inputs["attn_b"], np.float32)
    proj_w = np.asarray(inputs["proj_w"], np.float32)
    proj_b = np.asarray(inputs["proj_b"], np.float32)
    ln2_g = np.asarray(inputs["ln2_g"], np.float32)
    ln2_b = np.asarray(inputs["ln2_b"], np.float32)
    fc_w = np.asarray(inputs["fc_w"], np.float32)
    fc_b = np.asarray(inputs["fc_b"], np.float32)
    fc2_w = np.asarray(inputs["fc2_w"], np.float32)
    fc2_b = np.asarray(inputs["fc2_b"], np.float32)
    lnf_g = np.asarray(inputs["lnf_g"], np.float32)
    lnf_b = np.asarray(inputs["lnf_b"], np.float32)
    lm_w = np.asarray(inputs["lm_head_w"], np.float32)

    # fold LN affine into the following matmul
    wqkv_f = attn_w * ln1_g[:, None, :]                  # [L, 3D, D]
    bqkv_f = attn_b + np.einsum("lod,ld->lo", attn_w, ln1_b)
    wfc_f = fc_w * ln2_g[:, None, :]
    bfc_f = fc_b + np.einsum("lod,ld->lo", fc_w, ln2_b)
    wlm_f = lm_w * lnf_g[None, :]
    blm_f = lm_w @ lnf_b                                  # [V]

    wqkvT = wqkv_f.transpose(0, 2, 1)                     # [L, D, 3D]
    # pre-tiled [L, 3, P, KC, D] with slot order {K, V, Q}
    wq_t = wqkvT.reshape(L, KC, P, 3, D).transpose(0, 3, 2, 1, 4)
    wkvq = np.ascontiguousarray(_bf16(wq_t[:, [1, 2, 0]]))
    wprojT = np.ascontiguousarray(
        _bf16(proj_w.transpose(0, 2, 1).reshape(L, KC, P, D)
              .transpose(0, 2, 1, 3)))
    fp8m = bool(cfg.get("fp8_mlp"))
    _cvt = (lambda a: _fp8(a * FP8_WS)) if fp8m else _bf16
    wfcT = np.ascontiguousarray(
        _cvt(wfc_f.transpose(0, 2, 1).reshape(L, KC, P, F)
             .transpose(0, 2, 1, 3)))
    wfc2T = np.ascontiguousarray(
        _cvt(fc2_w.transpose(0, 2, 1).reshape(L, FC, P, D)
             .transpose(0, 2, 1, 3)))

    qkb = np.ascontiguousarray(bqkv_f[:, :2 * D])
    vb = np.ascontiguousarray(bqkv_f[:, 2 * D:])
    wlmT_full = _bf16(wlm_f.T)                            # [D, V]

    kk = np.arange(T, dtype=np.float64)
    slopes = 2.0 ** (-np.arange(1, H + 1, dtype=np.float64) * (ALIBI_BIAS_MAX / H))
    scale_f = 1.0 / math.sqrt(64)
    augk_t = np.stack([kk, np.ones(T)], axis=0).astype(np.float16)  # [2, T]

    in_maps = []
    for c in range(NCORES):
        r = c % GS
        tok = ids[c * NTOK:(c + 1) * NTOK]
        qg = (r * NTOK + np.arange(NTOK, dtype=np.float64))
        rel = kk[:, None] - qg[None, :]                  # k - q_glob
        # masked entries: cancel worst-case alibi (slope*rel <= smax) and
        # push the exp argument to ~-100
        smax = slopes[0] * T
        cm_t = np.where(rel <= 0, 0.0,
                        -(smax + 100.0) / scale_f).astype(np.float32)
        augq_t = np.stack([
            np.broadcast_to((slopes / scale_f)[:, None], (H, NTOK)),
            -(slopes[:, None] / scale_f) * qg[None, :],
        ], axis=0).astype(np.float16)                     # [2, H, NTOK]
        wlm_c = wlmT_full[:, c * VS:(c + 1) * VS]
        wlm_t = np.ascontiguousarray(
            wlm_c.reshape(KC, P, NVC2, PAIR * NV).transpose(2, 1, 0, 3))
        in_maps.append({
            "ids": np.ascontiguousarray(tok.reshape(-1, P)),
            "wte": wte,
            "wkvq": wkvq, "wprojT": wprojT, "wfcT": wfcT, "wfc2T": wfc2T,
            "qkb": qkb, "vb": vb, "pb": np.ascontiguousarray(proj_b),
            "fcb": bfc_f, "f2b": np.ascontiguousarray(fc2_b),
            "cm": cm_t, "augq": augq_t, "augk": augk_t,
            "wlmT": wlm_t,
            "lmb": np.ascontiguousarray(blm_f[c * VS:(c + 1) * VS]),
        })
    return in_maps


_NC_CACHE = {}


def biases_all_zero(in_maps):
    return all(
        not np.any(np.asarray(m[k], dtype=np.float32))
        for m in in_maps for k in ("vb", "pb", "f2b", "lmb", "qkb", "fcb"))


def run(cfg, inputs):
    from concourse.bass_utils import run_bass_kernel_spmd
    in_maps = prep_inputs(cfg, inputs)
    cfg = dict(cfg, zero_bias=biases_all_zero(in_maps))
    if cfg["zero_bias"]:
        for m in in_maps:
            for k in ("qkb", "vb", "pb", "fcb", "f2b", "lmb"):
                m.pop(k)
    key = tuple(sorted(cfg.items()))
    if key not in _NC_CACHE:
        _NC_CACHE[key] = build_program(cfg)
    nc = _NC_CACHE[key]
    res = run_bass_kernel_spmd(nc, in_maps, core_ids=list(range(NCORES)))
    outs = [np.asarray(res.results[c]["out"], dtype=np.float32)
            for c in range(NCORES)]
    B, T, V = cfg["B"], cfg["T"], cfg["V"]
    logits = np.concatenate(outs, axis=1).reshape(B, T, V)
    return logits


def kernel(**inputs) -> np.ndarray:
    return run(FULL, inputs)



# revision 27
# speedup vs baseline: 1.1252x; 1.1252x over previous
"""DolmaGPT (4-layer GPT, D=1024, H=16, T=1024, B=2, V=32000, ALiBi) on 8 TRN2 cores.

Strategy: sequence-parallel. Each core owns 256 token rows (cores 0-3 batch 0,
cores 4-7 batch 1). Weights replicated (bf16, streamed from HBM in pre-tiled
layouts so each load is one large-descriptor DMA). Per layer one fused 4-core
AllGather exchanges K^T and V together (bf16). lm_head vocab-sharded after an
8-core AllGather of the final hidden state. Residual stream fp32 in SBUF;
matmuls bf16 with fp32 PSUM accumulation.

v2 notes (instruction-count + pipelining oriented):
- Weights pre-tiled on host to [P, KC, F] so each load is one DMA with 2KB
  descriptors (v1 used 16-32 small strided DMAs per weight at 256B/desc).
- LN transposes via the DMA XBAR (dma_start_transpose) instead of PE
  transpose + copy chains.
- PSUM tiles span 4 banks' worth of chunks ([P, 4, NTOK]) so exp / gelu /
  PSUM->SBUF copies are one instruction per 4 matmul groups.
- K and V share one AllGather message per layer.
- lm_head processes vocab chunks in pairs per PSUM tile; output stored bf16.
- Softmax denominator broadcast on gpsimd (partition_broadcast), psy PSUM
  double-buffered: successive attention heads pipeline (this was worth ~2x
  on measured HW time).
- Output stores on the Act HWDGE queue. NOTE: gpsimd/SWDGE dma_start stores
  crashed the device (NRT_EXEC_UNIT_UNRECOVERABLE) alongside collectives.

Softmax: scores bounded, so no max-subtract. P = exp(scale*s) * M where
M = exp(alibi_bias) (0 where masked) is a precomputed per-core constant.
Denominator via ones-column appended to V (one extra PSUM row per head).
"""
import contextlib
import math
import numpy as np
import ml_dtypes

import concourse.bacc as bacc
import concourse.bass as bass
import concourse.mybir as mybir
import concourse.tile as tile
from concourse.bass import ts, ds
from concourse.masks import make_identity

P = 128
HD = 64
EPS = 1e-5
ALIBI_BIAS_MAX = 8.0
NCORES = 8
GS = 4  # AllGather group size for K/V (cores sharing one batch element)

FULL = dict(V=32000, D=1024, H=16, L=4, F=4096, B=2, T=1024,
            store_act=True)

F32 = mybir.dt.float32
BF16 = mybir.dt.bfloat16
FP8 = mybir.dt.float8e4
I32 = mybir.dt.int32
FP8_WS = 16.0   # fp8 MLP weight scale (values ~N(0,.02) -> normal range)


def build_program(cfg):
    V, D, H, L, F, B, T = (cfg[k] for k in ("V", "D", "H", "L", "F", "B", "T"))
    NTOK = B * T // NCORES      # tokens per core
    QT = NTOK // P              # q-token tiles per core
    KC = D // P                 # contract chunks over D
    KT = T // P                 # k-token tiles (attention keys, own batch)
    FT = F // P                 # MLP hidden tiles
    FC = F // P                 # fc2 contract chunks
    VS = V // NCORES            # vocab shard per core
    NV = 500 if VS % 1000 == 0 else (128 if VS % 256 == 0 else VS)
    NVC = VS // NV
    PAIR = 2 if NVC % 2 == 0 else 1
    NVC2 = NVC // PAIR          # fused weight chunks of PAIR*NV columns
    MT = B * T // P             # global token tiles (lm_head rows)
    scale = 1.0 / math.sqrt(HD)
    NCH = min(512, D)           # N-chunk for [tok, feat] matmuls
    WF = min(1024, D)           # feature width of one weight tile
    XX = D // NTOK if D >= NTOK else 0   # v-export row split (see kvmsg)
    H2 = NTOK // HD             # heads per 256-col row-chunk of v region
    ZB = bool(cfg.get("zero_bias"))  # skip K=1 bias matmuls when biases zero
    FP8M = bool(cfg.get("fp8_mlp"))  # fc/fc2 in fp8e4 DoubleRow (needs ZB)
    STORE_ACT = bool(cfg.get("store_act"))   # osb stores via Act HWDGE
    NO_XBAR = bool(cfg.get("no_xbar"))       # PE transposes instead of XBAR
    SP_DMA = bool(cfg.get("sp_dma"))         # all loads via SP queue
    Q4 = 4                      # chunk group per PSUM tile

    assert D % NTOK == 0 and H == (D // NTOK) * (NTOK // HD)
    assert H % 4 == 0 or H == 4

    nc = bacc.Bacc("TRN2", target_bir_lowering=False, debug=False,
                   num_devices=NCORES)
    eng2 = nc.sync if SP_DMA else nc.scalar   # gathered acts / masks queue

    # ---- DRAM parameters. All bf16 tensors live in ONE flat param: the
    # axon PJRT relay charges ~65us/call per parameter, so param count
    # dominates measured per-call time. Layout must match prep_inputs. ----
    assert not FP8M
    MDT = BF16
    # wte first: the indirect gather needs a zero-offset source AP
    SZ = dict(
        wte=V * D,
        wkvq=L * 3 * P * KC * D, wprojT=L * P * KC * D, wfcT=L * P * KC * F,
        wfc2T=L * P * FC * D, wlmT=NVC2 * P * KC * PAIR * NV,
        cm=T * NTOK, augk=3 * T, augq=3 * H * NTOK)
    OFS = {}
    _o = 0
    for _k, _n in SZ.items():
        OFS[_k] = _o
        _o += _n
    ids_in = nc.declare_dram_parameter("ids", [QT, P], I32, isOutput=False)
    wmega = nc.declare_dram_parameter("wmega", [_o], BF16, isOutput=False)

    def wv3(name, idx, d1, d2, d3):
        """3-d view [d1, d2, d3] of chunk idx of a mega sub-tensor."""
        o = OFS[name] + idx * d1 * d2 * d3
        return wmega[o:o + d1 * d2 * d3].rearrange(
            "(a b c) -> a b c", a=d1, b=d2)
    if not ZB:
        qkb_in = nc.declare_dram_parameter("qkb", [L, 2 * D], F32, isOutput=False)
        vb_in = nc.declare_dram_parameter("vb", [L, D], F32, isOutput=False)
        pb_in = nc.declare_dram_parameter("pb", [L, D], F32, isOutput=False)
        fcb_in = nc.declare_dram_parameter("fcb", [L, F], F32, isOutput=False)
        f2b_in = nc.declare_dram_parameter("f2b", [L, D], F32, isOutput=False)
        lmb_in = nc.declare_dram_parameter("lmb", [VS], F32, isOutput=False)
    out_d = nc.declare_dram_parameter("out", [B * T, VS], BF16, isOutput=True)

    HG = min(H, 4)              # heads per mask tile
    NHG = H // HG

    with tile.TileContext(nc) as tc:
        with (
            tc.tile_pool(name="const", bufs=1) as constp,
            tc.tile_pool(name="resident", bufs=1) as resp,
            tc.tile_pool(name="acts", bufs=1) as actp,
            tc.tile_pool(name="w", bufs=3) as wp_,
            tc.tile_pool(name="ln", bufs=2) as lnp,
            tc.tile_pool(name="stats", bufs=4) as statp,
            tc.tile_pool(name="mask", bufs=2) as maskp,
            tc.tile_pool(name="softmax", bufs=2) as softp,
            tc.tile_pool(name="outcp", bufs=2) as outp,
            tc.tile_pool(name="psA", bufs=(2 if NO_XBAR else 3),
                         space="PSUM") as psA,
            tc.tile_pool(name="psY", bufs=2, space="PSUM") as psY,
            tc.tile_pool(name="dram", bufs=2, space="DRAM") as dramp,
            contextlib.ExitStack() as estack,
        ):
            psT = (estack.enter_context(
                tc.tile_pool(name="psT", bufs=2, space="PSUM"))
                if NO_XBAR else None)
            # ---- constants ----
            if NO_XBAR:
                id_bf = constp.tile([P, P], BF16)
                make_identity(nc, id_bf[:, :])
            ones_bank = constp.tile([65, P], F32)
            nc.vector.memset(ones_bank[:, :], 1.0)
            eps_sb = constp.tile([P, 1], F32)
            nc.vector.memset(eps_sb[:, :], EPS)

            if not ZB:
                QKC = 2 * D // P
                qkb_sb = constp.tile([P, L * QKC], F32)
                nc.scalar.dma_start(
                    out=qkb_sb[:, :],
                    in_=qkb_in[:, :].rearrange("l (c p) -> p (l c)", p=P))
                fcb_sb = constp.tile([P, L * FT], F32)
                nc.scalar.dma_start(
                    out=fcb_sb[:, :],
                    in_=fcb_in[:, :].rearrange("l (c p) -> p (l c)", p=P))
                rows_sb = constp.tile([65, L * D], F32)
                nc.scalar.dma_start(
                    out=rows_sb[0:1, 0:L * D],
                    in_=vb_in[:, :].rearrange("l d -> (l d)")
                    .rearrange("(o f) -> o f", o=1))
                nc.scalar.dma_start(
                    out=rows_sb[32:33, 0:L * D],
                    in_=pb_in[:, :].rearrange("l d -> (l d)")
                    .rearrange("(o f) -> o f", o=1))
                nc.scalar.dma_start(
                    out=rows_sb[64:65, 0:L * D],
                    in_=f2b_in[:, :].rearrange("l d -> (l d)")
                    .rearrange("(o f) -> o f", o=1))
                vb_sb = rows_sb[0:1, :]
                pb_sb = rows_sb[32:33, :]
                f2b_sb = rows_sb[64:65, :]

            # ---- ALiBi folded into QK via 2 extra contraction rows:
            # psum += augk^T @ augq_h gives slope_h*(k - q)/scale exactly
            # (per-column f16 rounding cancels in softmax). Cm is 0 on
            # valid (k<=q) entries, -BIG on masked ones, added pre-exp. ----
            cm_sb = resp.tile([P, KT, NTOK], BF16)
            eng2.dma_start(out=cm_sb[:, :, :],
                           in_=wmega[OFS["cm"]:OFS["cm"] + T * NTOK]
                           .rearrange("(kc p q) -> p kc q", p=P, q=NTOK))
            augq_sb = resp.tile([3, H * NTOK], BF16)
            eng2.dma_start(out=augq_sb[:, :],
                           in_=wmega[OFS["augq"]:OFS["augq"] + 3 * H * NTOK]
                           .rearrange("(x hq) -> x hq", x=3))
            augk_sb = resp.tile([3, T], BF16)
            eng2.dma_start(out=augk_sb[:, :],
                           in_=wmega[OFS["augk"]:OFS["augk"] + 3 * T]
                           .rearrange("(x t) -> x t", x=3))

            # ---- residual stream x [p, a, d], fp32, resident;
            # wte rows gathered in bf16 then upcast ----
            x = resp.tile([P, QT, D], F32)
            xbf = actp.tile([P, QT, D], BF16, tag="xbf", bufs=1)
            wte_v = wmega[OFS["wte"]:OFS["wte"] + V * D].rearrange(
                "(v d) -> v d", v=V)
            for a in range(QT):
                ids_sb = statp.tile([P, 1], I32, tag="ids")
                nc.sync.dma_start(out=ids_sb[:, :],
                                  in_=ids_in[a, :].rearrange("(p o) -> p o", o=1))
                nc.gpsimd.indirect_dma_start(
                    out=xbf[:, a, :], out_offset=None,
                    in_=wte_v,
                    in_offset=bass.IndirectOffsetOnAxis(ap=ids_sb[:, 0:1], axis=0))
                nc.scalar.copy(out=x[:, a, :], in_=xbf[:, a, :])

            # ---- v_aug resident [p, kt, h*(HD+1)+j]; ones col per head ----
            v_aug = resp.tile([P, KT, H * (HD + 1)], BF16)
            va4 = v_aug.rearrange("p k (h j) -> p k h j", j=HD + 1)
            nc.vector.memset(va4[:, :, :, HD:HD + 1], 1.0)



            def layer_norm_T(src, zT, a_iter=None):
                """LN over free dim of src [P, QT, D] -> transposed bf16 zT
                [P, KC, NTOK] via the DMA XBAR."""
                for a in (range(QT) if a_iter is None else a_iter):
                    xs = src[:, a, :]
                    sm = statp.tile([P, 1], F32, tag="lnsm")
                    scr = lnp.tile([P, D], BF16, tag="lnscr", bufs=2)
                    nc.scalar.activation(scr[:, :], xs,
                                         mybir.ActivationFunctionType.Identity,
                                         accum_out=sm[:, :])
                    sqd = lnp.tile([P, D], BF16, tag="lnsqd", bufs=2)
                    sx2 = statp.tile([P, 1], F32, tag="lnsx2")
                    nc.scalar.activation(sqd[:, :], xs,
                                         mybir.ActivationFunctionType.Square,
                                         accum_out=sx2[:, :])
                    mean = statp.tile([P, 1], F32, tag="lnmean")
                    nc.scalar.mul(mean[:, :], sm[:, :], 1.0 / D)
                    nmean = statp.tile([P, 1], F32, tag="lnnmean")
                    nc.scalar.mul(nmean[:, :], sm[:, :], -1.0 / D)
                    b2 = statp.tile([P, 1], F32, tag="lnb2")
                    nc.vector.tensor_scalar(out=b2[:, :], in0=mean[:, :],
                                            scalar1=nmean[:, :],
                                            scalar2=eps_sb[:, :],
                                            op0=mybir.AluOpType.mult,
                                            op1=mybir.AluOpType.add)
                    std = statp.tile([P, 1], F32, tag="lnstd")
                    nc.scalar.activation(std[:, :], sx2[:, :],
                                         mybir.ActivationFunctionType.Sqrt,
                                         bias=b2[:, :], scale=1.0 / D)
                    rstd = statp.tile([P, 1], F32, tag="lnrstd")
                    nc.vector.reciprocal(rstd[:, :], std[:, :])
                    mrstd = statp.tile([P, 1], F32, tag="lnmrstd")
                    nc.vector.tensor_mul(out=mrstd[:, :], in0=mean[:, :],
                                         in1=rstd[:, :])
                    z = lnp.tile([P, D], BF16, tag="lnz", bufs=2)
                    nc.vector.tensor_scalar(out=z[:, :], in0=xs,
                                            scalar1=rstd[:, :],
                                            scalar2=mrstd[:, :],
                                            op0=mybir.AluOpType.mult,
                                            op1=mybir.AluOpType.subtract)
                    # zT[p, kc, t] = z[t, kc*128+p]
                    if NO_XBAR:
                        for kc in range(KC):
                            pt = psT.tile([P, P], BF16, tag="tr", bufs=2)
                            nc.tensor.transpose(out=pt[:, :],
                                                in_=z[:, ts(kc, P)],
                                                identity=id_bf[:, :])
                            nc.scalar.copy(out=zT[:, kc, ts(a, P)],
                                           in_=pt[:, :])
                    else:
                        nc.sync.dma_start_transpose(out=zT[:, :, ts(a, P)],
                                                    in_=z[:, :])

            def load_w(src_ap, name, dt=BF16):
                """One weight tile [P, KC, <=WF] from a pre-tiled layout.
                All tiles share one uniformly-shaped pool slot."""
                wt = wp_.tile([P, KC, WF], dt, tag="w", bufs=3, name=name)
                nc.sync.dma_start(out=wt[:, :, 0:src_ap.shape[2]], in_=src_ap)
                return wt

            def mm_quad(dst, wtile, h_src, c0, ncnk, l, bias_sb=None, boff=0,
                        act=None):
                """ncnk feature chunks [feat,tok] into one PSUM tile, then one
                batched copy/activation to dst[:, c0:c0+ncnk, :]."""
                ps = psA.tile([P, Q4, NTOK], F32, tag="quad")
                for j in range(ncnk):
                    for kc in range(KC):
                        nc.tensor.matmul(out=ps[:, j, :],
                                         lhsT=wtile[:, kc, ts(c0 + j, P)],
                                         rhs=h_src[:, kc, :],
                                         start=(kc == 0), stop=(kc == KC - 1))
                fn = act or mybir.ActivationFunctionType.Identity
                if ZB or bias_sb is None:
                    if act is None:
                        nc.scalar.copy(out=dst[:, c0:c0 + ncnk, :],
                                       in_=ps[:, 0:ncnk, :])
                    else:
                        nc.scalar.activation(dst[:, c0:c0 + ncnk, :],
                                             ps[:, 0:ncnk, :], fn)
                else:
                    for j in range(ncnk):
                        nc.scalar.activation(
                            dst[:, c0 + j, :], ps[:, j, :], fn,
                            bias=bias_sb[:, boff + c0 + j: boff + c0 + j + 1])

            nn0 = D // NCH
            AOFF = max(NCH, 512)    # bank-separate concurrent accum groups
            NKH = min(512, T)       # psum free width for kT_full
            assert QT == 2
            HPC = NCH // HD          # heads per v psum copy chunk
            for l in range(L):
                # ===== LN1 -> h1T, one token-half at a time; each half's
                # h is AllGathered separately so KV compute on half A
                # overlaps half B's collective =====
                h1T = actp.tile([P, KC, NTOK], BF16, tag="hT", bufs=1)
                SPLIT = bool(cfg.get("split_cc"))
                hgs = []
                if SPLIT:
                    for half in range(2):
                        layer_norm_T(x, h1T, a_iter=[half])
                        hmsg = dramp.tile([D, P], BF16, tag="kvmsg")
                        nc.sync.dma_start(
                            out=hmsg[:, :].rearrange("(kc p) q -> p kc q", p=P),
                            in_=h1T[:, :, ts(half, P)])
                        hg = dramp.tile([GS * D, P], BF16, tag="kvg")
                        if cfg.get("no_cc"):
                            for g in range(GS):
                                nc.sync.dma_start(out=hg[ts(g, D), :],
                                                  in_=hmsg[:, :])
                        else:
                            nc.gpsimd.collective_compute(
                                "AllGather", mybir.AluOpType.bypass,
                                ins=[hmsg[:, :].opt()], outs=[hg[:, :].opt()],
                                replica_groups=[[0, 1, 2, 3], [4, 5, 6, 7]])
                        hgs.append(hg)
                else:
                    layer_norm_T(x, h1T)
                    hmsg = dramp.tile([D, NTOK], BF16, tag="kvmsg")
                    nc.sync.dma_start(
                        out=hmsg[:, :].rearrange("(kc p) q -> p kc q", p=P),
                        in_=h1T[:, :, :])
                    hg = dramp.tile([GS * D, NTOK], BF16, tag="kvg")
                    if cfg.get("no_cc"):
                        for g in range(GS):
                            nc.sync.dma_start(out=hg[ts(g, D), :],
                                              in_=hmsg[:, :])
                    else:
                        nc.gpsimd.collective_compute(
                            "AllGather", mybir.AluOpType.bypass,
                            ins=[hmsg[:, :].opt()], outs=[hg[:, :].opt()],
                            replica_groups=[[0, 1, 2, 3], [4, 5, 6, 7]])

                wq = load_w(wv3("wkvq", l * 3 + 2, P, KC, D), f"wq{l}")
                wk = load_w(wv3("wkvq", l * 3 + 0, P, KC, D), f"wk{l}")
                wv = load_w(wv3("wkvq", l * 3 + 1, P, KC, D), f"wv{l}")

                # ===== qT (local tokens; overlaps the AllGathers) =====
                qT = actp.tile([P, KC, NTOK], BF16, tag="qT", bufs=1)
                for c0 in range(0, KC, Q4):
                    n = min(Q4, KC - c0)
                    mm_quad(qT, wq, h1T, c0, n, l,
                            bias_sb=None if ZB else qkb_sb,
                            boff=None if ZB else l * (2 * D // P))

                # ===== per half: unpack h, kT chunks, v tiles =====
                hg_sb = actp.tile([P, KC, T], BF16, tag="hg", bufs=1)
                hgv = hg_sb.rearrange("p k (g x q) -> p k g x q", x=2, q=P)
                kT = actp.tile([P, KC, T], BF16, tag="big32", bufs=1)
                kTv = kT.rearrange("p c (g x q) -> p c g x q", x=2, q=P)
                if not SPLIT:
                    for g in range(GS):
                        eng2.dma_start(
                            out=hg_sb[:, :, ts(g, NTOK)],
                            in_=hg[g * D:(g + 1) * D, :]
                            .rearrange("(kc p) q -> p kc q", p=P))
                for half in range(2):
                    if SPLIT:
                        for g in range(GS):
                            eng2.dma_start(
                                out=hgv[:, :, g, half, :],
                                in_=hgs[half][g * D:(g + 1) * D, :]
                                .rearrange("(kc p) q -> p kc q", p=P))
                    # kT chunks for this half's keys (all 4 cores' blocks)
                    for c in range(KC):
                        ps = psA.tile([P, Q4, NTOK], F32, tag="quad")
                        pso = ps.rearrange("p a q -> p (a q)")[:, 0:GS * P] \
                            .rearrange("p (g q) -> p g q", q=P)
                        for kc in range(KC):
                            nc.tensor.matmul(
                                out=pso[:, :, :],
                                lhsT=wk[:, kc, ts(c, P)],
                                rhs=hgv[:, kc, :, half, :],
                                start=(kc == 0), stop=(kc == KC - 1))
                        if ZB:
                            nc.scalar.copy(out=kTv[:, c, :, half, :],
                                           in_=pso[:, :, :])
                        else:
                            nc.scalar.activation(
                                kTv[:, c, :, half, :], pso[:, :, :],
                                mybir.ActivationFunctionType.Identity,
                                bias=qkb_sb[:, l * (2 * D // P) + KC + c:
                                            l * (2 * D // P) + KC + c + 1])
                    # v tiles for this half (jk % 2 == half)
                    for jk in range(half, KT, 2):
                        ps = psA.tile([P, Q4, NTOK], F32, tag="quad")
                        psf = ps.rearrange("p a q -> p (a q)")
                        for kc in range(KC):
                            for n0 in range(nn0):
                                nc.tensor.matmul(
                                    out=psf[:, n0 * AOFF:n0 * AOFF + NCH],
                                    lhsT=hg_sb[:, kc, ts(jk, P)],
                                    rhs=wv[:, kc, ts(n0, NCH)],
                                    start=(kc == 0),
                                    stop=(ZB and kc == KC - 1))
                        if not ZB:
                            for n0 in range(nn0):
                                nc.tensor.matmul(
                                    out=psf[:, n0 * AOFF:n0 * AOFF + NCH],
                                    lhsT=ones_bank[0:1, 0:P],
                                    rhs=vb_sb[:, l * D + n0 * NCH:
                                              l * D + n0 * NCH + NCH],
                                    start=False, stop=True)
                        for n0 in range(nn0):
                            nc.scalar.copy(
                                out=va4[:, jk, n0 * HPC:(n0 + 1) * HPC, 0:HD],
                                in_=psf[:, n0 * AOFF:n0 * AOFF + NCH]
                                .rearrange("p (h j) -> p h j", j=HD))

                wproj = load_w(wv3("wprojT", l, P, KC, D), f"wp{l}")
                nwf = (F + WF - 1) // WF  # fc weight tiles (feature-split)
                wfcs = [load_w(wv3("wfcT", l, P, KC, F)[:, :, ts(i, WF)], f"wfc{l}_{i}",
                               dt=MDT) for i in range(min(2, nwf))]

                # ===== attention per head =====
                yT = actp.tile([P, KC, NTOK], BF16, tag="yT", bufs=1)
                for h in range(H):
                    hr = (h % 2) * HD
                    hc = h // 2
                    p_all = softp.tile([P, KT, NTOK], BF16, tag="p_all", bufs=2)
                    for j0 in range(0, KT, Q4):
                        nq = min(Q4, KT - j0)
                        pss = psA.tile([P, Q4, NTOK], F32, tag="quad")
                        for jj in range(nq):
                            jk = j0 + jj
                            nc.tensor.matmul(
                                out=pss[:, jj, :],
                                lhsT=kT[hr:hr + HD, hc, ts(jk, P)],
                                rhs=qT[hr:hr + HD, hc, :],
                                start=True, stop=False)
                            nc.tensor.matmul(
                                out=pss[:, jj, :],
                                lhsT=augk_sb[0:3, ts(jk, P)],
                                rhs=augq_sb.rearrange("x (h q) -> x h q", q=NTOK)
                                [0:3, h, :],
                                start=False, stop=True)
                        nc.vector.tensor_add(out=pss[:, 0:nq, :],
                                             in0=pss[:, 0:nq, :],
                                             in1=cm_sb[:, j0:j0 + nq, :])
                        nc.scalar.activation(p_all[:, j0:j0 + nq, :],
                                             pss[:, 0:nq, :],
                                             mybir.ActivationFunctionType.Exp,
                                             scale=scale)
                    psy = psY.tile([HD + 1, NTOK], F32, tag="psy", bufs=2)
                    for jk in range(KT):
                        nc.tensor.matmul(
                            out=psy[:, :],
                            lhsT=v_aug[:, jk, h * (HD + 1):(h + 1) * (HD + 1)],
                            rhs=p_all[:, jk, :],
                            start=(jk == 0), stop=(jk == KT - 1))
                    rden = statp.tile([1, NTOK], F32, tag="rden")
                    nc.vector.reciprocal(rden[:, :], psy[HD:HD + 1, :])
                    rbc = softp.tile([HD, NTOK], F32, tag="rbc", bufs=2)
                    nc.gpsimd.partition_broadcast(rbc[:, :], rden[:, :],
                                                  channels=HD)
                    nc.vector.tensor_mul(out=yT[hr:hr + HD, hc, :],
                                         in0=psy[0:HD, :], in1=rbc[:, :])

                wfcs += [load_w(wv3("wfcT", l, P, KC, F)[:, :, ts(i, WF)], f"wfc{l}_{i}",
                                dt=MDT) for i in range(2, nwf)]

                # ===== proj + residual (lhsT reused across n0 chunks) =====
                for a in range(QT):
                    ps = psA.tile([P, Q4, NTOK], F32, tag="quad")
                    psf = ps.rearrange("p a q -> p (a q)")
                    for kc in range(KC):
                        for n0 in range(nn0):
                            nc.tensor.matmul(
                                out=psf[:, n0 * AOFF:n0 * AOFF + NCH],
                                lhsT=yT[:, kc, ts(a, P)],
                                rhs=wproj[:, kc, ts(n0, NCH)],
                                start=(kc == 0),
                                stop=(ZB and kc == KC - 1))
                    if not ZB:
                        for n0 in range(nn0):
                            nc.tensor.matmul(
                                out=psf[:, n0 * AOFF:n0 * AOFF + NCH],
                                lhsT=ones_bank[32:33, 0:P],
                                rhs=pb_sb[:, l * D + n0 * NCH:
                                          l * D + n0 * NCH + NCH],
                                start=False, stop=True)
                    if AOFF == NCH:
                        nc.vector.tensor_add(out=x[:, a, :], in0=x[:, a, :],
                                             in1=psf[:, 0:D])
                    else:
                        for n0 in range(nn0):
                            nc.vector.tensor_add(
                                out=x[:, a, ts(n0, NCH)],
                                in0=x[:, a, ts(n0, NCH)],
                                in1=psf[:, n0 * AOFF:n0 * AOFF + NCH])

                # ===== LN2 -> h2T; MLP =====
                h2T = actp.tile([P, KC, NTOK], BF16, tag="hT", bufs=1)
                layer_norm_T(x, h2T)

                if FP8M:
                    assert ZB and KC % 2 == 0
                    h2T8 = actp.tile([P, KC, NTOK], FP8, tag="qT8", bufs=1)
                    nc.scalar.copy(out=h2T8[:, :, :], in_=h2T[:, :, :])
                gT = actp.tile([P, FT, NTOK], MDT, tag="big32", bufs=1)
                FPW = WF // P  # feature chunks per fc tile
                FQ = min(Q4, FPW)
                DR = mybir.MatmulPerfMode.DoubleRow
                for c0 in range(0, FT, FQ):
                    n = min(FQ, FT - c0)
                    wt = wfcs[c0 // FPW]
                    ps = psA.tile([P, Q4, NTOK], F32, tag="quad")
                    for j in range(n):
                        loc = (c0 + j) - (c0 // FPW) * FPW
                        if FP8M:
                            for kp in range(KC // 2):
                                nc.tensor.matmul(
                                    out=ps[:, j, :],
                                    lhsT=wt[:, 2 * kp:2 * kp + 2, ts(loc, P)],
                                    rhs=h2T8[:, 2 * kp:2 * kp + 2, :],
                                    perf_mode=DR,
                                    start=(kp == 0), stop=(kp == KC // 2 - 1))
                        else:
                            for kc in range(KC):
                                nc.tensor.matmul(out=ps[:, j, :],
                                                 lhsT=wt[:, kc, ts(loc, P)],
                                                 rhs=h2T[:, kc, :],
                                                 start=(kc == 0),
                                                 stop=(kc == KC - 1))
                    gsc = (1.0 / FP8_WS) if FP8M else 1.0
                    if ZB:
                        nc.scalar.activation(gT[:, c0:c0 + n, :], ps[:, 0:n, :],
                                             mybir.ActivationFunctionType.Gelu,
                                             scale=gsc)
                    else:
                        for j in range(n):
                            nc.scalar.activation(
                                gT[:, c0 + j, :], ps[:, j, :],
                                mybir.ActivationFunctionType.Gelu,
                                bias=fcb_sb[:, l * FT + c0 + j:
                                            l * FT + c0 + j + 1])

                # ===== fc2: accumulate over FC chunks (lhsT reused over n0) =====
                FB = KC if FC % KC == 0 else min(8, FC)
                w2s = [load_w(wv3("wfc2T", l, P, FC, D)[:, ts(i, FB), :], f"wfc2{l}_{i}",
                              dt=MDT) for i in range(FC // FB)]
                q4s = [psA.tile([P, Q4, NTOK], F32, tag="quad",
                                name=f"fc2q{l}_{a}") for a in range(QT)]
                for fb in range(FC // FB):
                    w2 = w2s[fb]
                    for a in range(QT):
                        q4f = q4s[a].rearrange("p a q -> p (a q)")
                        if FP8M:
                            assert FB % 2 == 0
                            for jp in range(FB // 2):
                                j2 = fb * FB + 2 * jp
                                for n0 in range(nn0):
                                    nc.tensor.matmul(
                                        out=q4f[:, n0 * AOFF:n0 * AOFF + NCH],
                                        lhsT=gT[:, j2:j2 + 2, ts(a, P)],
                                        rhs=w2[:, 2 * jp:2 * jp + 2,
                                               ts(n0, NCH)],
                                        perf_mode=DR,
                                        start=(fb == 0 and jp == 0 and False
                                               or (fb == 0 and jp == 0)),
                                        stop=(fb == FC // FB - 1
                                              and jp == FB // 2 - 1))
                        else:
                            for j in range(FB):
                                for n0 in range(nn0):
                                    nc.tensor.matmul(
                                        out=q4f[:, n0 * AOFF:n0 * AOFF + NCH],
                                        lhsT=gT[:, fb * FB + j, ts(a, P)],
                                        rhs=w2[:, j, ts(n0, NCH)],
                                        start=(fb == 0 and j == 0),
                                        stop=(ZB and fb == FC // FB - 1
                                              and j == FB - 1))
                for a in range(QT):
                    q4f = q4s[a].rearrange("p a q -> p (a q)")
                    if not ZB:
                        for n0 in range(nn0):
                            nc.tensor.matmul(
                                out=q4f[:, n0 * AOFF:n0 * AOFF + NCH],
                                lhsT=ones_bank[64:65, 0:P],
                                rhs=f2b_sb[:, l * D + n0 * NCH:
                                           l * D + n0 * NCH + NCH],
                                start=False, stop=True)
                    if FP8M:
                        m2 = lnp.tile([P, D], F32, tag="m2", bufs=2)
                        nc.scalar.activation(
                            m2[:, :], q4f[:, 0:D],
                            mybir.ActivationFunctionType.Identity,
                            scale=1.0 / FP8_WS)
                        nc.vector.tensor_add(out=x[:, a, :], in0=x[:, a, :],
                                             in1=m2[:, :])
                    elif AOFF == NCH:
                        nc.vector.tensor_add(out=x[:, a, :], in0=x[:, a, :],
                                             in1=q4f[:, 0:D])
                    else:
                        for n0 in range(nn0):
                            nc.vector.tensor_add(
                                out=x[:, a, ts(n0, NCH)],
                                in0=x[:, a, ts(n0, NCH)],
                                in1=q4f[:, n0 * AOFF:n0 * AOFF + NCH])

            # ===== final LN -> xfT; AllGather over all 8 cores;
            # lm_head vocab-sharded (8MB weights/core) =====
            wl0 = load_w(wv3("wlmT", 0, P, KC, PAIR * NV), "wl0")
            xfT = actp.tile([P, KC, NTOK], BF16, tag="hT", bufs=1)
            layer_norm_T(x, xfT)
            xfmsg = dramp.tile([D, NTOK], BF16, tag="kvmsg")
            nc.sync.dma_start(
                out=xfmsg[:, :].rearrange("(kc p) q -> p kc q", p=P),
                in_=xfT[:, :, :])
            if cfg.get("no_cc"):
                xfg = dramp.tile([NCORES * D, NTOK], BF16, tag="xfg")
                for g in range(NCORES):
                    nc.sync.dma_start(out=xfg[ts(g, D), :], in_=xfmsg[:, :])
            else:
                xfg = dramp.tile([NCORES * D, NTOK], BF16, tag="xfg",
                                 addr_space="Shared")
                nc.gpsimd.collective_compute(
                    "AllGather", mybir.AluOpType.bypass,
                    ins=[xfmsg[:, :].opt()], outs=[xfg[:, :].opt()],
                    replica_groups=[list(range(NCORES))])
            # overlays the (now dead) gathered-h slot to fit SBUF
            xfg_sb = actp.tile([P, NCORES * KC, NTOK], BF16, tag="hg",
                               bufs=1)
            eng2.dma_start(
                out=xfg_sb[:, :, :],
                in_=xfg[:, :].rearrange("(c p) t -> p c t", p=P))

            for j in range(NVC2):
                wls = wl0 if j == 0 else load_w(wv3("wlmT", j, P, KC, PAIR * NV), f"wl{j}")
                if not ZB:
                    lmb_t = statp.tile([1, PAIR * NV], F32, tag="lmbt", bufs=2)
                    nc.scalar.dma_start(
                        out=lmb_t[:, :],
                        in_=lmb_in[j * PAIR * NV:(j + 1) * PAIR * NV]
                        .rearrange("(o v) -> o v", o=1))
                for m in range(MT):
                    g, i = m // QT, m % QT
                    ps = psA.tile([P, Q4, NTOK], F32, tag="quad")
                    psv = ps.rearrange("p a q -> p (a q)").rearrange(
                        "p (s v) -> p s v", s=PAIR)
                    for kc in range(KC):
                        for s in range(PAIR):
                            nc.tensor.matmul(
                                out=psv[:, s, 0:NV],
                                lhsT=xfg_sb[:, g * KC + kc, ts(i, P)],
                                rhs=wls[:, kc, s * NV:s * NV + NV],
                                start=(kc == 0),
                                stop=(ZB and kc == KC - 1))
                    if not ZB:
                        for s in range(PAIR):
                            nc.tensor.matmul(
                                out=psv[:, s, 0:NV],
                                lhsT=ones_bank[0:1, 0:P],
                                rhs=lmb_t[:, s * NV:(s + 1) * NV],
                                start=False, stop=True)
                    osb = outp.tile([P, PAIR, NV], BF16, tag="osb",
                                    bufs=(1 if NO_XBAR else 2))
                    nc.scalar.copy(out=osb[:, :, :], in_=psv[:, :, 0:NV])
                    (nc.scalar if STORE_ACT else nc.gpsimd).dma_start(
                        out=out_d[ts(m, P), j * PAIR * NV:(j + 1) * PAIR * NV],
                        in_=osb[:, :, :])

    nc.finalize()
    return nc


# ---------------- host side ----------------

def _bf16(a):
    return np.asarray(a, dtype=ml_dtypes.bfloat16)


def _fp8(a):
    return np.asarray(a, dtype=ml_dtypes.float8_e4m3)


def prep_inputs(cfg, inputs):
    """Build the 8 per-core input maps from full inputs (pre-tiled layouts)."""
    V, D, H, L, F, B, T = (cfg[k] for k in ("V", "D", "H", "L", "F", "B", "T"))
    NTOK = B * T // NCORES
    KC = D // P
    FC = F // P
    VS = V // NCORES
    NV = 500 if VS % 1000 == 0 else (128 if VS % 256 == 0 else VS)
    NVC = VS // NV
    PAIR = 2 if NVC % 2 == 0 else 1
    NVC2 = NVC // PAIR

    ids = np.asarray(inputs["input_ids"]).astype(np.int32).reshape(-1)
    wte = np.asarray(inputs["wte"], dtype=np.float32)
    ln1_g = np.asarray(inputs["ln1_g"], np.float32)
    ln1_b = np.asarray(inputs["ln1_b"], np.float32)
    attn_w = np.asarray(inputs["attn_w"], np.float32)
    attn_b = np.asarray(inputs["attn_b"], np.float32)
    proj_w = np.asarray(inputs["proj_w"], np.float32)
    proj_b = np.asarray(inputs["proj_b"], np.float32)
    ln2_g = np.asarray(inputs["ln2_g"], np.float32)
    ln2_b = np.asarray(inputs["ln2_b"], np.float32)
    fc_w = np.asarray(inputs["fc_w"], np.float32)
    fc_b = np.asarray(inputs["fc_b"], np.float32)
    fc2_w = np.asarray(inputs["fc2_w"], np.float32)
    fc2_b = np.asarray(inputs["fc2_b"], np.float32)
    lnf_g = np.asarray(inputs["lnf_g"], np.float32)
    lnf_b = np.asarray(inputs["lnf_b"], np.float32)
    lm_w = np.asarray(inputs["lm_head_w"], np.float32)

    # fold LN affine into the following matmul
    wqkv_f = attn_w * ln1_g[:, None, :]                  # [L, 3D, D]
    bqkv_f = attn_b + np.einsum("lod,ld->lo", attn_w, ln1_b)
    wfc_f = fc_w * ln2_g[:, None, :]
    bfc_f = fc_b + np.einsum("lod,ld->lo", fc_w, ln2_b)
    wlm_f = lm_w * lnf_g[None, :]
    blm_f = lm_w @ lnf_b                                  # [V]

    wqkvT = wqkv_f.transpose(0, 2, 1)                     # [L, D, 3D]
    # pre-tiled [L, 3, P, KC, D] with slot order {K, V, Q}
    wq_t = wqkvT.reshape(L, KC, P, 3, D).transpose(0, 3, 2, 1, 4)
    wkvq = np.ascontiguousarray(_bf16(wq_t[:, [1, 2, 0]]))
    wprojT = np.ascontiguousarray(
        _bf16(proj_w.transpose(0, 2, 1).reshape(L, KC, P, D)
              .transpose(0, 2, 1, 3)))
    wfcT = np.ascontiguousarray(
        _bf16(wfc_f.transpose(0, 2, 1).reshape(L, KC, P, F)
              .transpose(0, 2, 1, 3)))
    wfc2T = np.ascontiguousarray(
        _bf16(fc2_w.transpose(0, 2, 1).reshape(L, FC, P, D)
              .transpose(0, 2, 1, 3)))

    qkb = np.ascontiguousarray(bqkv_f[:, :2 * D])
    vb = np.ascontiguousarray(bqkv_f[:, 2 * D:])
    wlmT_full = _bf16(wlm_f.T)                            # [D, V]

    kk = np.arange(T, dtype=np.float64)
    slopes = 2.0 ** (-np.arange(1, H + 1, dtype=np.float64) * (ALIBI_BIAS_MAX / H))
    scale_f = 1.0 / math.sqrt(64)
    augk_t = np.stack([kk // 4, kk % 4, np.ones(T)],
                      axis=0).astype(ml_dtypes.bfloat16)  # [3, T]
    wte_b = _bf16(wte)

    in_maps = []
    for c in range(NCORES):
        r = c % GS
        tok = ids[c * NTOK:(c + 1) * NTOK]
        qg = (r * NTOK + np.arange(NTOK, dtype=np.float64))
        rel = kk[:, None] - qg[None, :]                  # k - q_glob
        # masked entries: cancel worst-case alibi (slope*rel <= smax) and
        # push the exp argument to ~-100
        smax = slopes[0] * T
        cm_t = np.where(rel <= 0, 0.0,
                        -(smax + 100.0) / scale_f).astype(ml_dtypes.bfloat16)
        # 3-row aug (all bf16-exact): k = 4*k_hi + k_lo
        augq_t = np.stack([
            np.broadcast_to((4.0 * slopes / scale_f)[:, None], (H, NTOK)),
            np.broadcast_to((slopes / scale_f)[:, None], (H, NTOK)),
            -(slopes[:, None] / scale_f) * qg[None, :],
        ], axis=0).astype(ml_dtypes.bfloat16)             # [3, H, NTOK]
        wlm_c = wlmT_full[:, c * VS:(c + 1) * VS]
        wlm_t = np.ascontiguousarray(
            wlm_c.reshape(KC, P, NVC2, PAIR * NV).transpose(2, 1, 0, 3))
        wmega = np.concatenate([
            np.ascontiguousarray(a).ravel() for a in
            (wte_b, wkvq, wprojT, wfcT, wfc2T, wlm_t, cm_t, augk_t, augq_t)])
        in_maps.append({
            "ids": np.ascontiguousarray(tok.reshape(-1, P)),
            "wmega": wmega,
            "qkb": qkb, "vb": vb, "pb": np.ascontiguousarray(proj_b),
            "fcb": bfc_f, "f2b": np.ascontiguousarray(fc2_b),
            "lmb": np.ascontiguousarray(blm_f[c * VS:(c + 1) * VS]),
        })
    return in_maps


_NC_CACHE = {}


def biases_all_zero(in_maps):
    return all(
        not np.any(np.asarray(m[k], dtype=np.float32))
        for m in in_maps for k in ("vb", "pb", "f2b", "lmb", "qkb", "fcb"))


def run(cfg, inputs):
    from concourse.bass_utils import run_bass_kernel_spmd
    in_maps = prep_inputs(cfg, inputs)
    cfg = dict(cfg, zero_bias=biases_all_zero(in_maps))
    if cfg["zero_bias"]:
        for m in in_maps:
            for k in ("qkb", "vb", "pb", "fcb", "f2b", "lmb"):
                m.pop(k)
    key = tuple(sorted(cfg.items()))
    if key not in _NC_CACHE:
        _NC_CACHE[key] = build_program(cfg)
    nc = _NC_CACHE[key]
    res = run_bass_kernel_spmd(nc, in_maps, core_ids=list(range(NCORES)))
    outs = [np.asarray(res.results[c]["out"], dtype=np.float32)
            for c in range(NCORES)]
    B, T, V = cfg["B"], cfg["T"], cfg["V"]
    logits = np.concatenate(outs, axis=1).reshape(B, T, V)
    return logits


def kernel(**inputs) -> np.ndarray:
    return run(FULL, inputs)

